# revision 14
# baseline (speedup 1.0000x reference)
# Trainium2 Bass kernel for BertNER head:
#   out = softmax(compact_valid(x) @ W + b)
#
# Math trick: compact_valid is a per-batch-row permutation-with-zero-fill,
# i.e. a matrix P with one-hot rows (dest p picks the (p+1)-th valid token)
# and zero rows for the padded tail.  Since softmax(0*W + b) == softmax(b),
#   out = softmax(P @ (X @ W) + b)
# so the ragged gather becomes a small matmul on the [S, 9] logits.
# P is built on-device: r = cumsum(mask)*mask (1-indexed rank of each valid
# token, 0 for invalid; DVE tensor_tensor_scan does the cumsum), and
# P^T[s, p] = (r[s] == iota[p]) via a DVE compare against an iota constant.
# The iota column order is interleaved (token 4*p + j in compare-tile j,
# partition p) so each output partition holds 4 consecutive tokens and the
# final DMA writes 144B-contiguous runs instead of 36B ones.
#
# Sharding: pure data parallel over the batch dim, 8 rows per core.
# Per core: X [4096, 1024] f32 read (16MB -> the memory roofline driver),
# cast to bf16 during the SWDGE DMA with partition-major token staging
# (token = 4p + n) so each partition reads one contiguous 16KB run.  The
# X^T transposes run on the PE in
# f32 "pair-packed" form (one f32 word = two adjacent-h bf16 values;
# transpose mode is pure routing, so bit patterns survive) which halves the
# PE instruction count.  Z^T = W^T @ X^T on PE (bf16, N=512 streams, W
# host-permuted to match the packed h order), small PE transposes back to
# Z[s,9] bf16, compaction C = P@Z in bf16 (FWL), softmax on ACT/DVE
# (logits are tiny: |z| < ~5, so no max-subtraction).

import numpy as np
import ml_dtypes

B, S, H, L = 64, 512, 1024, 9
NCORES = 8
BL = B // NCORES      # batch rows per core
T = BL * S            # tokens per core
P = 128
HC = H // P           # 8 h-chunks
HC2 = HC // 2         # 4 pair-packed super-chunks (256 h each)
NSC = S // P          # 4 s-chunks per batch row
JW = S // P           # tokens per partition in the interleaved output (4)

_cache = {}


def _build(reps=1, dma_only=False, tdt_name='float32', split_loads=False, xin_bufs=3, pmaj=True, gpair=True):
    import concourse.bass as bass
    import concourse.mybir as mybir
    import concourse.tile as tile
    from concourse import bacc

    f32 = mybir.dt.float32
    bf16 = mybir.dt.bfloat16
    i32 = mybir.dt.int32
    i16 = mybir.dt.int16

    nc = bacc.Bacc(
        "TRN2",
        target_bir_lowering=False,
        debug=False,
        enable_asserts=False,
        num_devices=NCORES,
    )

    x = nc.dram_tensor("x", (T, H), f32, kind="ExternalInput").ap()
    mask = nc.dram_tensor("mask", (BL, S), i32, kind="ExternalInput").ap()
    w = nc.dram_tensor("w", (H, L), f32, kind="ExternalInput").ap()
    bb = nc.dram_tensor("b_bcast", (P, L), f32, kind="ExternalInput").ap()
    iota1 = nc.dram_tensor("iota1", (P, S), i16, kind="ExternalInput").ap()
    idf = nc.dram_tensor("id_f32", (P, P), f32, kind="ExternalInput").ap()
    out = nc.dram_tensor("out", (T, L), f32, kind="ExternalOutput").ap()

    AL = mybir.AluOpType
    AF = mybir.ActivationFunctionType
    tdt = getattr(mybir.dt, tdt_name)

    with tile.TileContext(nc) as tc:
        with (
            tc.tile_pool(name="consts", bufs=1) as cpool,
            tc.tile_pool(name="xin", bufs=xin_bufs) as xpool,
            tc.tile_pool(name="xt", bufs=8) as xtpool,
            tc.tile_pool(name="pt", bufs=8) as ptpool,
            tc.tile_pool(name="z", bufs=3) as zpool,
            tc.tile_pool(name="small", bufs=3) as spool,
            tc.tile_pool(name="outp", bufs=2) as opool,
            tc.tile_pool(name="pst", bufs=4, space="PSUM") as pst,
            tc.tile_pool(name="psacc", bufs=4, space="PSUM") as psacc,
        ):
            # ---- constants ----
            id_f = cpool.tile([P, P], f32)
            nc.sync.dma_start(id_f, idf)
            id_t = cpool.tile([P, P], tdt)
            nc.sync.dma_start(id_t, idf.bitcast(tdt))
            iota_sb = cpool.tile([P, S], i16)
            nc.sync.dma_start(iota_sb, iota1)
            bb_sb = cpool.tile([P, L], f32)
            nc.sync.dma_start(bb_sb, bb)
            # W host-permuted: chunk ko holds rows h = 256*(ko//2)+2k+(ko%2)
            w_sb = cpool.tile([P, HC, L], bf16)
            nc.gpsimd.dma_start(w_sb, w.rearrange("(ko ki) l -> ki ko l", ki=P))

            for _rep in range(reps):
                # ---- r = cumsum(mask)*mask (1-indexed rank, 0 if invalid) ----
                mask_sb = spool.tile([BL, S], i32, name="mask_sb", tag="mask")
                nc.sync.dma_start(mask_sb, mask)
                maskf = spool.tile([BL, S], f32, name="maskf", tag="maskf")
                nc.vector.tensor_copy(out=maskf, in_=mask_sb)
                cums = spool.tile([BL, S], f32, name="cums", tag="cums")
                nc.vector.tensor_tensor_scan(
                    cums, maskf, maskf, 0.0, AL.add, AL.bypass
                )
                rrow = spool.tile([BL, S], f32, name="rrow", tag="rrow")
                nc.vector.tensor_tensor(out=rrow, in0=cums, in1=maskf, op=AL.mult)
                rT = spool.tile([P, NSC, BL], f32, name="rT", tag="rT")
                if pmaj:
                    rrow_r = rrow.rearrange("b (q n) -> b n q", n=NSC)
                for sc in range(NSC):
                    rtp = psacc.tile([P, BL], f32, name="rtp", tag="acc")
                    nc.tensor.matmul(
                        rtp,
                        rrow_r[:, sc, :] if pmaj else rrow[:, sc * P : (sc + 1) * P],
                        id_f[:BL, :BL],
                        is_transpose=True,
                        start=True,
                        stop=True,
                    )
                    nc.vector.tensor_copy(out=rT[:, sc, :], in_=rtp)

                if dma_only:
                    dummy = opool.tile([P, JW, L], f32, name="dummy", tag="outt")
                    nc.vector.tensor_copy(out=dummy, in_=bb_sb[:, 0:1].to_broadcast((P, JW, L)))
                    for g in range(BL):
                        nc.sync.dma_start(
                            out[g * S : (g + 1) * S, :].rearrange("(p j) l -> p j l", j=JW),
                            dummy,
                        )
                # ---- main loop over batch rows (512 tokens each) ----
                xg2 = None
                for g in range(BL):
                    if gpair:
                        # one 4MB cast-DMA covers two batch rows
                        if g % 2 == 0:
                            xg2 = xpool.tile(
                                [P, 2, NSC, H], bf16, name="xg2", tag="xg"
                            )
                            nc.gpsimd.dma_start(
                                xg2,
                                x[g * S : (g + 2) * S, :].rearrange(
                                    "(gi p n) h -> p gi n h", gi=2, n=NSC
                                ),
                            )
                        xg = xg2[:, g % 2]
                    else:
                        xg = xpool.tile([P, NSC, H], bf16, name="xg", tag="xg")
                    if gpair:
                        pass
                    elif split_loads:
                        half = S // 2
                        for hfi in range(2):
                            nc.gpsimd.dma_start(
                                xg[:, hfi * (NSC // 2) : (hfi + 1) * (NSC // 2), :],
                                x[
                                    g * S + hfi * half : g * S + (hfi + 1) * half, :
                                ].rearrange("(n p) h -> p n h", p=P),
                            )
                    elif pmaj:
                        # token = 4p + n: one contiguous 16KB read per partition
                        nc.gpsimd.dma_start(
                            xg,
                            x[g * S : (g + 1) * S, :].rearrange("(p n) h -> p n h", n=NSC),
                        )
                    else:
                        nc.gpsimd.dma_start(
                            xg,
                            x[g * S : (g + 1) * S, :].rearrange("(n p) h -> p n h", p=P),
                        )
                    if dma_only:
                        continue
                    # 4-byte view: one word = two adjacent-h bf16 values
                    xgf = xg.bitcast(tdt)  # [128, 4, 512]

                    # pair-packed transpose: [128t, 128pair] -> [128pair, 512t]
                    xts = []
                    for hc2 in range(HC2):
                        ps = pst.tile([P, S], tdt, name="ps", tag="pst")
                        for n in range(NSC):
                            nc.tensor.matmul(
                                ps[:, n * P : (n + 1) * P],
                                xgf[:, n, hc2 * P : (hc2 + 1) * P],
                                id_t,
                                is_transpose=True,
                                start=True,
                                stop=True,
                            )
                        xt = xtpool.tile([P, S], tdt, name="xt", tag="xt")
                        if hc2 % 2 == 0:
                            nc.scalar.copy(out=xt, in_=ps)
                        else:
                            nc.vector.tensor_copy(out=xt, in_=ps)
                        xts.append(xt)

                    # Z^T = W^T @ X^T -> [9, 512] f32 (one psum bank)
                    zTp = psacc.tile([L, S], f32, name="zTp", tag="acc")
                    for ko in range(HC):
                        hc2, parity = ko // 2, ko % 2
                        xv = xts[hc2].bitcast(bf16)  # [128, 1024], free=(t,parity)
                        nc.tensor.matmul(
                            zTp,
                            w_sb[:, ko, :],
                            xv[:, parity :: 2],
                            start=(ko == 0),
                            stop=(ko == HC - 1),
                        )
                    zTs = zpool.tile([L, S], f32, name="zTs", tag="zTs")
                    nc.scalar.copy(out=zTs, in_=zTp)

                    # small transposes back to Z [s, 9] per s-chunk (bf16 out)
                    zt = zpool.tile([P, NSC, L], bf16, name="zt", tag="zt")
                    for n in range(NSC):
                        zfp = psacc.tile([P, L], f32, name="zfp", tag="acc")
                        nc.tensor.matmul(
                            zfp,
                            zTs[:, n * P : (n + 1) * P],
                            id_f[:L, :L],
                            is_transpose=True,
                            start=True,
                            stop=True,
                        )
                        nc.vector.tensor_copy(out=zt[:, n, :], in_=zfp)

                    # P^T tiles: PT[s, col] = (r[s] == iota[col]); interleaved
                    # col order: col j*128+p corresponds to dest token 4p+j
                    pts = []
                    for sc in range(NSC):
                        pt_t = ptpool.tile([P, S], bf16, name="pt_t", tag="pt")
                        nc.vector.tensor_scalar(
                            pt_t,
                            iota_sb,
                            rT[:, sc, g : g + 1],
                            None,
                            AL.is_equal,
                        )
                        pts.append(pt_t)

                    # C = P @ Z (accumulate all s-chunks) + bias
                    cb = spool.tile([P, NSC, L], f32, name="cb", tag="cb")
                    for j in range(JW):
                        cp = psacc.tile([P, L], f32, name="cp", tag="acc")
                        for sc in range(NSC):
                            nc.tensor.matmul(
                                cp,
                                pts[sc][:, j * P : (j + 1) * P],
                                zt[:, sc, :],
                                start=(sc == 0),
                                stop=(sc == NSC - 1),
                            )
                        nc.vector.tensor_tensor(
                            out=cb[:, j, :], in0=cp, in1=bb_sb, op=AL.add
                        )

                    # softmax over the last dim (9), batched over the 4 tiles
                    e_t = spool.tile([P, JW, L], f32, name="e_t", tag="e")
                    nc.scalar.activation(e_t, cb, AF.Exp)
                    es = spool.tile([P, JW], f32, name="es", tag="es")
                    nc.vector.reduce_sum(es, e_t, axis=mybir.AxisListType.X)
                    ri = spool.tile([P, JW], f32, name="ri", tag="ri")
                    nc.vector.reciprocal(ri, es)
                    outt = opool.tile([P, JW, L], f32, name="outt", tag="outt")
                    nc.vector.tensor_tensor(
                        out=outt,
                        in0=e_t,
                        in1=ri[:, :, None].to_broadcast((P, JW, L)),
                        op=AL.mult,
                    )
                    # partition p holds tokens 4p..4p+3 -> 144B contiguous runs
                    nc.sync.dma_start(
                        out[g * S : (g + 1) * S, :].rearrange(
                            "(p j) l -> p j l", j=JW
                        ),
                        outt,
                    )

    nc.compile()
    return nc


def _get_nc():
    if "nc" not in _cache:
        _cache["nc"] = _build()
    return _cache["nc"]


def _make_in_maps(sequence_output, valid_mask, W, b):
    xs = np.ascontiguousarray(np.asarray(sequence_output), dtype=np.float32)
    mk = np.ascontiguousarray(np.asarray(valid_mask), dtype=np.int32)
    Wf = np.asarray(W, dtype=np.float32)
    bf = np.asarray(b, dtype=np.float32)

    # permute W rows to the pair-packed h order: chunk ko (0..7) covers
    # h = 256*(ko//2) + 2k + (ko%2), k = 0..127
    hidx = np.empty(H, dtype=np.int64)
    kk = np.arange(P)
    for ko in range(HC):
        hidx[ko * P : (ko + 1) * P] = 256 * (ko // 2) + 2 * kk + (ko % 2)
    w_perm = np.ascontiguousarray(Wf[hidx])

    # interleaved iota: compare-tile j, partition p <-> dest token 4p+j,
    # matched against r = 1-indexed rank (so value 4p+j+1)
    iota_np = np.empty((S,), dtype=np.int16)
    for j in range(JW):
        iota_np[j * P : (j + 1) * P] = 4 * np.arange(P) + j + 1
    iota_np = np.ascontiguousarray(np.broadcast_to(iota_np, (P, S)))

    idf_np = np.eye(P, dtype=np.float32)
    bb_np = np.ascontiguousarray(np.broadcast_to(bf, (P, L)))

    in_maps = []
    for c in range(NCORES):
        in_maps.append(
            {
                "x": xs[c * BL : (c + 1) * BL].reshape(T, H),
                "mask": mk[c * BL : (c + 1) * BL],
                "w": w_perm,
                "b_bcast": bb_np,
                "iota1": iota_np,
                "id_f32": idf_np,
            }
        )
    return in_maps


def kernel(sequence_output, valid_mask, W, b):
    from concourse.bass_utils import run_bass_kernel_spmd

    nc = _get_nc()
    in_maps = _make_in_maps(sequence_output, valid_mask, W, b)
    res = run_bass_kernel_spmd(nc, in_maps, core_ids=list(range(NCORES)))
    _cache["last_results"] = res

    outs = [res.results[c]["out"].reshape(BL, S, L) for c in range(NCORES)]
    return np.concatenate(outs, axis=0).astype(np.float32)
